# revision 12
# baseline (speedup 1.0000x reference)
"""Trainium2 Bass kernel for nn_MultiOmicGATModule (3-layer hetero GATv2 + matmul tail).

Strategy (8 NeuronCores, SPMD single NEFF):
 - Gene nodes dst-sharded: core k owns rows [2500k, 2500k+2500). Edges are
   routed to the core owning their destination, sorted by dst, and tiled into
   127-node tiles (slot 127 = trash for padding).
 - Per layer: dense hs tables (src-transformed features) are computed
   replicated on every core (cpg table from host-pretransposed c^T); hd tables
   only for the core's own 2500 rows.
 - Edge aggregation is vertex-centric: per 128-edge chunk, a one-hot
   membership matrix (DVE is_equal vs iota constant) is built; PE matmuls
   expand destination features (hd) and segment-sum the per-edge payload
   [ex*hs | ex | pad] into a PSUM accumulator per relation -> numerator and
   softmax denominator in one pass. Segment-max is skipped (logits are in
   [-10, 8]; softmax is shift-invariant).
 - Update: combine relations, ELU (shift-invariant form), residual, LayerNorm
   on own rows; AllGather replicates new gene features for the next layer.
 - Tail: batch matmuls contraction-sharded over nodes + one AllReduce, then
   LayerNorm.
"""
import math
import numpy as np

import concourse.bass as bass
import concourse.bacc as bacc
import concourse.tile as tile
from concourse import mybir
from concourse.bass_utils import run_bass_kernel_spmd
from concourse.bass_interp import get_hw_module

F32 = mybir.dt.float32
I16 = mybir.dt.int16
P = 128

FULL_CFG = dict(
    N_GENE=20000, N_CPG=50000, N_MIR=2000, B=64, NCORES=8,
    TILE_NODES=127, IDX_LIM=32768, N_LAYERS=3,
)


def _roundup(x, m):
    return (x + m - 1) // m * m


def wrap_idx16(a):
    """[L] int -> [128, L//16] int16 wrapped layout, replicated 8x across gpsimd cores."""
    L = a.shape[0]
    assert L % 16 == 0
    w = a.reshape(L // 16, 16).T.astype(np.int16)
    return np.ascontiguousarray(np.tile(w, (8, 1)))


def slot_layout(a):
    """[L] -> [128, L//128] f32, token e at [e%128, e//128]."""
    L = a.shape[0]
    assert L % P == 0
    return np.ascontiguousarray(a.reshape(L // P, P).T.astype(np.float32))


def host_prep(inputs, cfg):
    """Build per-core in_maps + the static chunk-count config."""
    NG, NCPG, NMIR = cfg['N_GENE'], cfg['N_CPG'], cfg['N_MIR']
    NC = cfg['NCORES']
    TN = cfg['TILE_NODES']
    NPC = NG // NC
    NT = math.ceil(NPC / TN)
    IDX_LIM = cfg['IDX_LIM']
    NL = cfg['N_LAYERS']

    pa = inputs['params']
    f = lambda x: np.ascontiguousarray(np.asarray(x, np.float32))
    c = f(pa['emb_cpg']); mi = f(pa['emb_mir']); g0 = f(pa['emb_gene'])

    sl = np.arange(NG, dtype=np.int64)
    edges = {
        'cg': (np.asarray(inputs['cg_src'], np.int64), np.asarray(inputs['cg_dst'], np.int64)),
        'mg': (np.asarray(inputs['mg_src'], np.int64), np.asarray(inputs['mg_dst'], np.int64)),
        'gg': (np.concatenate([np.asarray(inputs['gg_src'], np.int64), sl]),
               np.concatenate([np.asarray(inputs['gg_dst'], np.int64), sl])),
    }
    GROUPS = [('cglo', 'cg'), ('cghi', 'cg'), ('mg', 'mg'), ('gg', 'gg')]

    # route / sort / tile / split
    per_core = {}   # (k, grp) -> list over t of (gidx array, slot array)
    for k in range(NC):
        for rel in ['cg', 'mg', 'gg']:
            s, d = edges[rel]
            selm = (d // NPC) == k
            ss, dd = s[selm], d[selm] - k * NPC
            o = np.argsort(dd, kind='stable')
            ss, dd = ss[o], dd[o]
            tid = dd // TN
            slot = dd - tid * TN
            for t in range(NT):
                m = tid == t
                st, so = ss[m], slot[m]
                if rel == 'cg':
                    lo = st < IDX_LIM
                    per_core[(k, 'cglo', t)] = (st[lo], so[lo])
                    per_core[(k, 'cghi', t)] = (st[~lo] - IDX_LIM, so[~lo])
                else:
                    per_core[(k, rel, t)] = (st, so)

    # equalize chunk counts across cores
    nch = {}   # (grp, t) -> chunks of 128
    for grp, rel in GROUPS:
        for t in range(NT):
            mx = max(per_core[(k, grp, t)][0].shape[0] for k in range(NC))
            nch[(grp, t)] = _roundup(max(mx, 1), P) // P

    # pack per-group arrays per core
    gi = {k: {} for k in range(NC)}
    slo = {k: {} for k in range(NC)}
    for grp, rel in GROUPS:
        for k in range(NC):
            gparts, sparts = [], []
            for t in range(NT):
                cnt = nch[(grp, t)] * P
                a, b = per_core[(k, grp, t)]
                ap = np.zeros(cnt, np.int64); ap[:a.shape[0]] = a
                bp = np.full(cnt, 127, np.int64); bp[:b.shape[0]] = b
                gparts.append(ap); sparts.append(bp)
            gi[k][grp] = wrap_idx16(np.concatenate(gparts))
            slo[k][grp] = slot_layout(np.concatenate(sparts))

    # constants
    iota = np.tile(np.arange(P, dtype=np.float32)[None, :], (P, 1))
    ident = np.eye(P, dtype=np.float32)

    NCPGS, NMIRS = NCPG // NC, NMIR // NC
    xg = f(inputs['xg']); xc = f(inputs['xc']); xm = f(inputs['xm'])
    xgT = np.ascontiguousarray(xg.T) / np.float32(math.sqrt(NG))
    xcT = np.ascontiguousarray(xc.T) / np.float32(math.sqrt(NCPG))
    xmT = np.ascontiguousarray(xm.T) / np.float32(math.sqrt(NMIR))

    in_maps = []
    for k in range(NC):
        m = {
            'cT': f(c.T.reshape(2, 128, NCPG)),
            'miT': f(mi.T.reshape(2, 128, NMIR)),
            'g_full0': g0,
            'g_own0': f(g0[k * NPC:(k + 1) * NPC]),
            'g_own0T': f(g0[k * NPC:(k + 1) * NPC].T.reshape(2, 128, NPC)),
            'iota': iota, 'ident': ident,
            'xgT': f(xgT[k * NPC:(k + 1) * NPC]),
            'xcT': f(xcT[k * NCPGS:(k + 1) * NCPGS]),
            'xmT': f(xmT[k * NMIRS:(k + 1) * NMIRS]),
            'c_rows': f(c[k * NCPGS:(k + 1) * NCPGS]),
            'mi_rows': f(mi[k * NMIRS:(k + 1) * NMIRS]),
            'on_g_g': f(pa['on_gene'][0]), 'on_g_b': f(pa['on_gene'][1]),
            'on_c_g': f(pa['on_cpg'][0]), 'on_c_b': f(pa['on_cpg'][1]),
            'on_m_g': f(pa['on_mir'][0]), 'on_m_b': f(pa['on_mir'][1]),
        }
        for grp, rel in GROUPS:
            m[f'gi_{grp}'] = gi[k][grp]
            m[f'slot_{grp}'] = slo[k][grp]
        for L in range(NL):
            lp = pa['layers'][L]
            bias_sum = np.zeros(256, np.float32)
            for rel in ['cg', 'mg', 'gg']:
                rp = lp[rel]
                m[f'Wl_{L}_{rel}'] = f(np.asarray(rp['Wl']).reshape(2, 128, 256))
                m[f'Wr_{L}_{rel}'] = f(np.asarray(rp['Wr']).reshape(2, 128, 256))
                m[f'bl_{L}_{rel}'] = f(rp['bl'])
                m[f'br_{L}_{rel}'] = f(rp['br'])
                m[f'att_{L}_{rel}'] = f(np.asarray(rp['att']).reshape(256))
                bias_sum += np.asarray(rp['bias'], np.float32)
            m[f'bias_sum_{L}'] = bias_sum
            m[f'ln_g_{L}'] = f(lp['ln_g'])
            m[f'ln_b_{L}'] = f(lp['ln_b'])
        in_maps.append(m)

    cfg2 = dict(cfg)
    cfg2['nch'] = nch
    cfg2['GROUPS'] = GROUPS
    cfg2['NPC'] = NPC
    cfg2['NT'] = NT
    return in_maps, cfg2


def bcast_part(ap, parts=P):
    """DRAM [D] vector -> broadcast AP over partitions [parts, D]."""
    return bass.AP(tensor=ap.tensor, offset=ap.offset, ap=[[0, parts]] + list(ap.ap))


def bfree(ap2d, n, axis_pos=1):
    """[P, D] AP -> [P, n, D] with 0-step broadcast middle dim."""
    a = list(ap2d.ap)
    return bass.AP(tensor=ap2d.tensor, offset=ap2d.offset, ap=[a[0], [0, n]] + a[1:])


def build_program(cfg):
    NG, NCPG, NMIR = cfg['N_GENE'], cfg['N_CPG'], cfg['N_MIR']
    NC, B, NL = cfg['NCORES'], cfg['B'], cfg['N_LAYERS']
    TN, NPC, NT = cfg['TILE_NODES'], cfg['NPC'], cfg['NT']
    IDX_LIM = cfg['IDX_LIM']
    nch = cfg['nch']
    GROUPS = cfg['GROUPS']
    NCPGS, NMIRS = NCPG // NC, NMIR // NC
    HDROWS = _roundup(TN * (NT - 1) + P, 64)
    EPS = 1e-5
    DEN_EPS = 1e-10

    nc = bacc.Bacc("TRN2", target_bir_lowering=False, debug=False, num_devices=NC)
    dt = lambda n, s, d=F32, k="ExternalInput": nc.dram_tensor(n, s, d, kind=k).ap()

    cT = dt('cT', [2, 128, NCPG]); miT = dt('miT', [2, 128, NMIR])
    g_full0 = dt('g_full0', [NG, 256])
    g_own0 = dt('g_own0', [NPC, 256]); g_own0T = dt('g_own0T', [2, 128, NPC])
    iota_t = dt('iota', [P, P]); ident_t = dt('ident', [P, P])
    xgT = dt('xgT', [NPC, B]); xcT = dt('xcT', [NCPGS, B]); xmT = dt('xmT', [NMIRS, B])
    c_rows = dt('c_rows', [NCPGS, 256]); mi_rows = dt('mi_rows', [NMIRS, 256])
    on_ap = {n: dt(n, [256]) for n in ['on_g_g', 'on_g_b', 'on_c_g', 'on_c_b', 'on_m_g', 'on_m_b']}
    gi_ap, slot_ap, tot = {}, {}, {}
    for grp, rel in GROUPS:
        T = sum(nch[(grp, t)] for t in range(NT)) * P
        tot[grp] = T
        gi_ap[grp] = dt(f'gi_{grp}', [128, T // 16], I16)
        slot_ap[grp] = dt(f'slot_{grp}', [128, T // P])
    wap = {}
    for L in range(NL):
        for rel in ['cg', 'mg', 'gg']:
            for w in ['Wl', 'Wr']:
                wap[(w, L, rel)] = dt(f'{w}_{L}_{rel}', [2, 128, 256])
            for v in ['bl', 'br', 'att']:
                wap[(v, L, rel)] = dt(f'{v}_{L}_{rel}', [256])
        for v in ['bias_sum', 'ln_g', 'ln_b']:
            wap[(v, L)] = dt(f'{v}_{L}', [256])

    # internal DRAM
    it = lambda n, s, d=F32: nc.dram_tensor(n, s, d).ap()
    hs_tab = {'cg': it('hs_cg', [NCPG, 256]), 'mg': it('hs_mg', [NMIR, 256]),
              'gg': it('hs_gg', [NG, 256])}
    hd_tab = {r: it(f'hd_{r}', [HDROWS, 256]) for r in ['cg', 'mg', 'gg']}
    g_own_buf = [g_own0] + [it(f'g_own{L + 1}', [NPC, 256]) for L in range(NL)]
    g_ownT_buf = [g_own0T] + [it(f'g_ownT{L + 1}', [2, 128, NPC]) for L in range(NL - 1)]
    ag_out = [None] + [nc.dram_tensor(f'ag_out{L + 1}', [NG, 256], F32, addr_space="Shared").ap()
                       for L in range(NL - 1)]
    ar_in = it('ar_in', [B, 3, 256])
    ar_out = nc.dram_tensor('ar_out', [B, 3, 256], F32, addr_space="Shared").ap()
    zout = dt('zout', [3, B, 256], F32, "ExternalOutput")

    SEGCH = min(8, max(nch.values()))

    with tile.TileContext(nc) as tc:
        with tc.tile_pool(name="consts", bufs=1) as consts, \
             tc.tile_pool(name="wpool", bufs=1) as wpool, \
             tc.tile_pool(name="tabs", bufs=3) as tabs, \
             tc.tile_pool(name="edges", bufs=2) as epool, \
             tc.tile_pool(name="upd", bufs=2) as upool, \
             tc.tile_pool(name="psA", bufs=2, space="PSUM") as psA, \
             tc.tile_pool(name="psB", bufs=2, space="PSUM") as psB, \
             tc.tile_pool(name="psR", bufs=1, space="PSUM") as psR:

            iota_sb = consts.tile([P, P], F32)
            nc.sync.dma_start(out=iota_sb[:], in_=iota_t)
            ident_sb = consts.tile([P, P], F32)
            nc.sync.dma_start(out=ident_sb[:], in_=ident_t)
            eps_sb = consts.tile([P, 1], F32)
            nc.vector.memset(eps_sb[:], EPS)
            zero_sb = consts.tile([P, 256], F32)
            nc.vector.memset(zero_sb[:], 0.0)
            # zero hd table pad rows once
            for r in ['cg', 'mg', 'gg']:
                if HDROWS > NPC:
                    nc.sync.dma_start(out=hd_tab[r][NPC:HDROWS, :], in_=zero_sb[:HDROWS - NPC, :])

            _rep_n = [0]

            def rep_tile(vec_ap, tag=None):
                if tag is None:
                    _rep_n[0] += 1
                    tag = f"rep{_rep_n[0]}"
                t = wpool.tile([P, 256], F32, tag=tag)
                nc.sync.dma_start(out=t[:], in_=bcast_part(vec_ap))
                return t

            def table_rowmajor(dst_ap, n_rows, srcT_ap, W_ap, bias_rep, src_is_dram):
                """dst[n_rows,256] = srcT.T @ W + bias. srcT: [2,128,n_rows] DRAM."""
                ntile = math.ceil(n_rows / P)
                for t in range(ntile):
                    w = min(P, n_rows - t * P)
                    lt = tabs.tile([P, 2, P], F32, tag="tab_lhsT")
                    nc.sync.dma_start(
                        out=lt[:, :, :w],
                        in_=srcT_ap.rearrange("k p n -> p k n")[:, :, t * P:t * P + w])
                    ps = psB.tile([P, 256], F32, space="PSUM", tag="psB")
                    for kk in range(2):
                        nc.tensor.matmul(ps[:w, :], lhsT=lt[:, kk, :w], rhs=W_ap[:, kk, :],
                                         start=(kk == 0), stop=(kk == 1), skip_group_check=True)
                    ot = tabs.tile([P, 256], F32, tag="tab_out")
                    nc.vector.tensor_tensor(out=ot[:w, :], in0=ps[:w, :], in1=bias_rep[:w, :],
                                            op=mybir.AluOpType.add)
                    nc.sync.dma_start(out=dst_ap[t * P:t * P + w, :], in_=ot[:w, :])

            def table_from_rows(dst_ap, n_rows, src_rows_ap, W_ap, bias_rep):
                """dst[n_rows,256] = src_rows @ W + bias, transposing tiles on PE."""
                ntile = math.ceil(n_rows / P)
                for t in range(ntile):
                    w = min(P, n_rows - t * P)
                    rt = tabs.tile([P, 256], F32, tag="tab_rows")
                    nc.sync.dma_start(out=rt[:w, :], in_=src_rows_ap[t * P:t * P + w, :])
                    gt = tabs.tile([P, 2, P], F32, tag="tab_gT")
                    for kk in range(2):
                        pst = psA.tile([P, P], F32, space="PSUM", tag="psA")
                        nc.tensor.transpose(out=pst[:, :w], in_=rt[:w, kk * 128:(kk + 1) * 128],
                                            identity=ident_sb[:w, :w])
                        nc.scalar.copy(out=gt[:, kk, :w], in_=pst[:, :w])
                    ps = psB.tile([P, 256], F32, space="PSUM", tag="psB")
                    for kk in range(2):
                        nc.tensor.matmul(ps[:w, :], lhsT=gt[:, kk, :w], rhs=W_ap[:, kk, :],
                                         start=(kk == 0), stop=(kk == 1), skip_group_check=True)
                    ot = tabs.tile([P, 256], F32, tag="tab_out")
                    nc.vector.tensor_tensor(out=ot[:w, :], in0=ps[:w, :], in1=bias_rep[:w, :],
                                            op=mybir.AluOpType.add)
                    nc.sync.dma_start(out=dst_ap[t * P:t * P + w, :], in_=ot[:w, :])

            # ---------------- layers ----------------
            for L in range(NL):
                g_full = g_full0 if L == 0 else ag_out[L]
                g_ownT = g_ownT_buf[L]

                att_rep, W_sb = {}, {}
                for rel in ['cg', 'mg', 'gg']:
                    att_rep[rel] = rep_tile(wap[('att', L, rel)], tag=f"att_{rel}")
                    for w in ['Wl', 'Wr']:
                        t_ = wpool.tile([P, 2, 256], F32, tag=f"{w}_{rel}")
                        nc.sync.dma_start(out=t_[:], in_=wap[(w, L, rel)].rearrange("k p n -> p k n"))
                        W_sb[(w, rel)] = t_
                bias_sum_rep = rep_tile(wap[('bias_sum', L)], tag="bias_sum")
                lng_rep = rep_tile(wap[('ln_g', L)], tag="lng")
                lnb_rep = rep_tile(wap[('ln_b', L)], tag="lnb")
                bl_rep = {rel: rep_tile(wap[('bl', L, rel)], tag=f"bl_{rel}") for rel in ['cg', 'mg', 'gg']}
                br_rep = {rel: rep_tile(wap[('br', L, rel)], tag=f"br_{rel}") for rel in ['cg', 'mg', 'gg']}

                # hd tables (own rows only)
                for rel in ['cg', 'mg', 'gg']:
                    table_rowmajor(hd_tab[rel], NPC, g_ownT, W_sb[('Wr', rel)], br_rep[rel], True)
                # hs tables
                table_rowmajor(hs_tab['cg'], NCPG, cT, W_sb[('Wl', 'cg')], bl_rep['cg'], True)
                table_rowmajor(hs_tab['mg'], NMIR, miT, W_sb[('Wl', 'mg')], bl_rep['mg'], True)
                table_from_rows(hs_tab['gg'], NG, g_full, W_sb[('Wl', 'gg')], bl_rep['gg'])

                gsrc = {'cglo': hs_tab['cg'][0:min(IDX_LIM, NCPG), :],
                        'cghi': hs_tab['cg'][IDX_LIM:NCPG, :] if NCPG > IDX_LIM else None,
                        'mg': hs_tab['mg'], 'gg': hs_tab['gg']}

                goff = {grp: 0 for grp, _ in GROUPS}
                for t in range(NT):
                    nodes_t = min(TN, NPC - TN * t)
                    hdt = {}
                    for rel in ['cg', 'mg', 'gg']:
                        h = upool.tile([P, 256], F32, tag=f"hdt_{rel}")
                        nc.sync.dma_start(out=h[:], in_=hd_tab[rel][TN * t:TN * t + P, :])
                        hdt[rel] = h
                    psum_r = {rel: psR.tile([P, 320], F32, space="PSUM", tag=f"seg_{rel}",
                                            name=f"psum_seg_{rel}_{L}_{t}")
                              for rel in ['cg', 'mg', 'gg']}
                    started = {rel: False for rel in ['cg', 'mg', 'gg']}
                    lastgrp = {}
                    for grp, rel in GROUPS:
                        if nch[(grp, t)] > 0 and gsrc[grp] is not None:
                            lastgrp[rel] = grp

                    for grp, rel in GROUPS:
                        n_total = nch[(grp, t)]
                        if n_total == 0 or gsrc[grp] is None:
                            continue
                        nseg = math.ceil(n_total / SEGCH)
                        for si in range(nseg):
                            n = min(SEGCH, n_total - si * SEGCH)
                            last_seg = si == nseg - 1
                            off = goff[grp]
                            goff[grp] = off + n
                            ne = n * P
                            gixsb = epool.tile([P, SEGCH * 8], I16, tag="gix")
                            nc.sync.dma_start(out=gixsb[:, :ne // 16],
                                              in_=gi_ap[grp][:, off * 8:off * 8 + ne // 16])
                            slsb = epool.tile([P, SEGCH], F32, tag="slot")
                            nc.sync.dma_start(out=slsb[:, :n], in_=slot_ap[grp][:, off:off + n])
                            hs_big = epool.tile([P, SEGCH, 256], F32, tag="hs_big")
                            nc.gpsimd.dma_gather(
                                out_ap=hs_big[:, :n, :], in_ap=gsrc[grp], idxs_ap=gixsb[:, :ne // 16],
                                num_idxs=ne, num_idxs_reg=ne, elem_size=256)
                            oh_w = epool.tile([P, SEGCH, P], F32, tag="oh_w")
                            s_w = epool.tile([P, SEGCH, 256], F32, tag="s_w")
                            pay = epool.tile([P, SEGCH, 320], F32, tag="pay")
                            for j in range(n):
                                slotb = bass.AP(tensor=slsb.tensor, offset=slsb[:, j:j + 1].offset,
                                                ap=[slsb[:].ap[0], [0, P]])
                                nc.vector.tensor_tensor(
                                    out=oh_w[:, j, :], in0=slotb,
                                    in1=iota_sb[:], op=mybir.AluOpType.is_equal)
                                pst = psA.tile([P, P], F32, space="PSUM", tag="psA")
                                nc.tensor.transpose(out=pst[:], in_=oh_w[:, j, :], identity=ident_sb[:])
                                ohT = epool.tile([P, P], F32, tag="ohT")
                                nc.scalar.copy(out=ohT[:], in_=pst[:])
                                psh = psB.tile([P, 256], F32, space="PSUM", tag="psB")
                                nc.tensor.matmul(psh[:], lhsT=ohT[:], rhs=hdt[rel][:],
                                                 start=True, stop=True, skip_group_check=True)
                                nc.vector.tensor_tensor(out=s_w[:, j, :], in0=hs_big[:, j, :],
                                                        in1=psh[:], op=mybir.AluOpType.add)
                            # batched elementwise over the segment's chunks
                            # (pay[:, :, 0:256] doubles as the 0.2*s scratch)
                            nc.scalar.mul(out=pay[:, :n, 0:256], in_=s_w[:, :n, :], mul=0.2)
                            nc.vector.tensor_tensor(out=s_w[:, :n, :], in0=s_w[:, :n, :],
                                                    in1=pay[:, :n, 0:256], op=mybir.AluOpType.max)
                            nc.vector.tensor_tensor(out=s_w[:, :n, :], in0=s_w[:, :n, :],
                                                    in1=bfree(att_rep[rel][:], n),
                                                    op=mybir.AluOpType.mult)
                            e_w = epool.tile([P, SEGCH, 8], F32, tag="e_w")
                            nc.vector.reduce_sum(out=e_w[:, :n, :],
                                                 in_=s_w[:, :n, :].rearrange("p c (h k) -> p c h k", k=32),
                                                 axis=mybir.AxisListType.X)
                            ex_w = epool.tile([P, SEGCH, 8], F32, tag="ex_w")
                            nc.scalar.activation(out=ex_w[:, :n, :], in_=e_w[:, :n, :],
                                                 func=mybir.ActivationFunctionType.Exp)
                            exb = bass.AP(tensor=ex_w.tensor, offset=ex_w[:].offset,
                                          ap=[ex_w[:].ap[0], [8, n], [1, 8], [0, 32]])
                            nc.vector.tensor_tensor(
                                out=pay[:, :n, 0:256].rearrange("p c (h k) -> p c h k", k=32),
                                in0=hs_big[:, :n, :].rearrange("p c (h k) -> p c h k", k=32),
                                in1=exb, op=mybir.AluOpType.mult)
                            nc.vector.tensor_copy(out=pay[:, :n, 256:264], in_=ex_w[:, :n, :])
                            nc.vector.memset(pay[:, :n, 264:320], 0.0)
                            for j in range(n):
                                st = not started[rel]
                                started[rel] = True
                                sp = (grp == lastgrp[rel]) and last_seg and (j == n - 1)
                                nc.tensor.matmul(psum_r[rel][:], lhsT=oh_w[:, j, :], rhs=pay[:, j, :],
                                                 start=st, stop=sp, skip_group_check=True)

                    # ---- update own nodes of tile t ----
                    o_acc = upool.tile([P, 256], F32, tag="o_acc")
                    tmp = upool.tile([P, 256], F32, tag="tmp")
                    first = True
                    for rel in ['cg', 'mg', 'gg']:
                        den = upool.tile([P, 8], F32, tag="den")
                        nc.vector.tensor_scalar(out=den[:], in0=psum_r[rel][:, 256:264],
                                                scalar1=DEN_EPS, scalar2=None,
                                                op0=mybir.AluOpType.add)
                        rec = upool.tile([P, 8], F32, tag="rec")
                        nc.vector.reciprocal(out=rec[:], in_=den[:])
                        recb = bass.AP(tensor=rec.tensor, offset=rec[:].offset,
                                       ap=[rec[:].ap[0], [1, 8], [0, 32]])
                        dst = o_acc if first else tmp
                        nc.vector.tensor_tensor(
                            out=dst[:].rearrange("p (h k) -> p h k", k=32),
                            in0=psum_r[rel][:, 0:256].rearrange("p (h k) -> p h k", k=32),
                            in1=recb, op=mybir.AluOpType.mult)
                        if not first:
                            nc.vector.tensor_tensor(out=o_acc[:], in0=o_acc[:], in1=tmp[:],
                                                    op=mybir.AluOpType.add)
                        first = False
                    nc.vector.tensor_tensor(out=o_acc[:], in0=o_acc[:], in1=bias_sum_rep[:],
                                            op=mybir.AluOpType.add)
                    # elu(o) + 1 = relu(o) + exp(min(o,0)); the -1 shift cancels in LN
                    neg = upool.tile([P, 256], F32, tag="neg")
                    nc.vector.tensor_scalar(out=neg[:], in0=o_acc[:], scalar1=0.0, scalar2=None,
                                            op0=mybir.AluOpType.min)
                    en = upool.tile([P, 256], F32, tag="en")
                    nc.scalar.activation(out=en[:], in_=neg[:], func=mybir.ActivationFunctionType.Exp)
                    nc.vector.tensor_scalar(out=o_acc[:], in0=o_acc[:], scalar1=0.0, scalar2=None,
                                            op0=mybir.AluOpType.max)
                    nc.vector.tensor_tensor(out=o_acc[:], in0=o_acc[:], in1=en[:],
                                            op=mybir.AluOpType.add)
                    gold = upool.tile([P, 256], F32, tag="gold")
                    nc.sync.dma_start(out=gold[:nodes_t, :],
                                      in_=g_own_buf[L][TN * t:TN * t + nodes_t, :])
                    nc.vector.tensor_tensor(out=o_acc[:nodes_t, :], in0=o_acc[:nodes_t, :],
                                            in1=gold[:nodes_t, :], op=mybir.AluOpType.add)
                    # LayerNorm
                    stats = upool.tile([P, 6], F32, tag="stats")
                    nc.vector.bn_stats(out=stats[:nodes_t, :], in_=o_acc[:nodes_t, :])
                    mv = upool.tile([P, 2], F32, tag="mv")
                    nc.vector.bn_aggr(out=mv[:nodes_t, :], in_=stats[:nodes_t, :])
                    sd = upool.tile([P, 1], F32, tag="sd")
                    nc.scalar.activation(out=sd[:nodes_t, :], in_=mv[:nodes_t, 1:2],
                                         func=mybir.ActivationFunctionType.Sqrt,
                                         bias=eps_sb[:nodes_t, :])
                    rs = upool.tile([P, 1], F32, tag="rs")
                    nc.vector.reciprocal(out=rs[:nodes_t, :], in_=sd[:nodes_t, :])
                    nc.vector.tensor_scalar(out=o_acc[:nodes_t, :], in0=o_acc[:nodes_t, :],
                                            scalar1=mv[:nodes_t, 0:1], scalar2=rs[:nodes_t, :],
                                            op0=mybir.AluOpType.subtract,
                                            op1=mybir.AluOpType.mult)
                    nc.vector.tensor_tensor(out=o_acc[:nodes_t, :], in0=o_acc[:nodes_t, :],
                                            in1=lng_rep[:nodes_t, :], op=mybir.AluOpType.mult)
                    nc.vector.tensor_tensor(out=o_acc[:nodes_t, :], in0=o_acc[:nodes_t, :],
                                            in1=lnb_rep[:nodes_t, :], op=mybir.AluOpType.add)
                    nc.sync.dma_start(out=g_own_buf[L + 1][TN * t:TN * t + nodes_t, :],
                                      in_=o_acc[:nodes_t, :])
                    if L < NL - 1:
                        gtn = upool.tile([P, 2, P], F32, tag="gtn")
                        for kk in range(2):
                            pst = psA.tile([P, P], F32, space="PSUM", tag="psA")
                            nc.tensor.transpose(out=pst[:, :nodes_t],
                                                in_=o_acc[:nodes_t, kk * 128:(kk + 1) * 128],
                                                identity=ident_sb[:nodes_t, :nodes_t])
                            nc.scalar.copy(out=gtn[:, kk, :nodes_t], in_=pst[:, :nodes_t])
                            nc.sync.dma_start(
                                out=g_ownT_buf[L + 1].rearrange("k p n -> p k n")[:, kk, TN * t:TN * t + nodes_t],
                                in_=gtn[:, kk, :nodes_t])

                if L < NL - 1:
                    nc.gpsimd.collective_compute(
                        "AllGather", mybir.AluOpType.bypass,
                        replica_groups=[list(range(NC))],
                        ins=[g_own_buf[L + 1].opt()], outs=[ag_out[L + 1].opt()])

            # ---------------- tail ----------------
            def tail_mm(xT_ap, rows_ap, n_rows):
                ps = psB.tile([P, 256], F32, space="PSUM", tag="psB")
                ntile = math.ceil(n_rows / P)
                for t in range(ntile):
                    w = min(P, n_rows - t * P)
                    lt = upool.tile([P, B], F32, tag="tail_lt")
                    nc.sync.dma_start(out=lt[:w, :], in_=xT_ap[t * P:t * P + w, :])
                    rt = upool.tile([P, 256], F32, tag="tail_rt")
                    nc.sync.dma_start(out=rt[:w, :], in_=rows_ap[t * P:t * P + w, :])
                    nc.tensor.matmul(ps[:B, :], lhsT=lt[:w, :B], rhs=rt[:w, :],
                                     start=(t == 0), stop=(t == ntile - 1), skip_group_check=True)
                return ps

            zcat = upool.tile([P, 3, 256], F32, tag="zcat")
            for i, (xT_ap, rows_ap, n_rows) in enumerate([
                    (xgT, g_own_buf[NL], NPC), (xcT, c_rows, NCPGS), (xmT, mi_rows, NMIRS)]):
                ps = tail_mm(xT_ap, rows_ap, n_rows)
                nc.scalar.copy(out=zcat[:B, i, :], in_=ps[:B, :])
            nc.sync.dma_start(out=ar_in, in_=zcat[:B, :, :])
            nc.gpsimd.collective_compute(
                "AllReduce", mybir.AluOpType.add,
                replica_groups=[list(range(NC))],
                ins=[ar_in.opt()], outs=[ar_out.opt()])
            zs = upool.tile([P, 3, 256], F32, tag="zs")
            nc.sync.dma_start(out=zs[:B, :, :], in_=ar_out)
            for i, (gn, bn) in enumerate([('on_g_g', 'on_g_b'), ('on_c_g', 'on_c_b'),
                                          ('on_m_g', 'on_m_b')]):
                x = zs[:B, i, :]
                stats = upool.tile([P, 6], F32, tag="stats")
                nc.vector.bn_stats(out=stats[:B, :], in_=x)
                mv = upool.tile([P, 2], F32, tag="mv")
                nc.vector.bn_aggr(out=mv[:B, :], in_=stats[:B, :])
                sd = upool.tile([P, 1], F32, tag="sd")
                nc.scalar.activation(out=sd[:B, :], in_=mv[:B, 1:2],
                                     func=mybir.ActivationFunctionType.Sqrt, bias=eps_sb[:B, :])
                rs = upool.tile([P, 1], F32, tag="rs")
                nc.vector.reciprocal(out=rs[:B, :], in_=sd[:B, :])
                nc.vector.tensor_scalar(out=x, in0=x, scalar1=mv[:B, 0:1], scalar2=rs[:B, :],
                                        op0=mybir.AluOpType.subtract, op1=mybir.AluOpType.mult)
                grt = rep_tile(on_ap[gn])
                brt = rep_tile(on_ap[bn])
                nc.vector.tensor_tensor(out=x, in0=x, in1=grt[:B, :], op=mybir.AluOpType.mult)
                nc.vector.tensor_tensor(out=x, in0=x, in1=brt[:B, :], op=mybir.AluOpType.add)
                nc.sync.dma_start(out=zout[i, :, :], in_=zs[:B, i, :])

    nc.compile()
    return nc


_last_results = None


def build(inputs, cfg_overrides=None):
    cfg = dict(FULL_CFG)
    if cfg_overrides:
        cfg.update(cfg_overrides)
    in_maps, cfg = host_prep(inputs, cfg)
    nc = build_program(cfg)
    nc.m = get_hw_module(nc.m)
    return nc, in_maps, cfg


def kernel(**inputs):
    global _last_results
    nc, in_maps, cfg = build(inputs)
    res = run_bass_kernel_spmd(nc, in_maps, core_ids=list(range(cfg['NCORES'])))
    _last_results = res
    z = np.asarray(res.results[0]['zout']).reshape(3, cfg['B'], 256)
    return np.ascontiguousarray(z[0]), np.ascontiguousarray(z[1]), np.ascontiguousarray(z[2])


# revision 35
# speedup vs baseline: 1.2373x; 1.2373x over previous
"""Trainium2 Bass kernel for nn_MultiOmicGATModule (3-layer hetero GATv2 + matmul tail).

Strategy (8 NeuronCores, SPMD single NEFF):
 - Gene nodes dst-sharded: core k owns rows [2500k, 2500k+2500). Edges are
   routed to the core owning their destination, sorted by dst, and tiled into
   127-node tiles (slot 127 = trash for padding).
 - Per layer: dense hs tables (src-transformed features) are computed
   replicated on every core (cpg table from host-pretransposed c^T); hd tables
   only for the core's own 2500 rows.
 - Edge aggregation is vertex-centric: per 128-edge chunk, a one-hot
   membership matrix (DVE is_equal vs iota constant) is built; PE matmuls
   expand destination features (hd) and segment-sum the per-edge payload
   [ex*hs | ex | pad] into a PSUM accumulator per relation -> numerator and
   softmax denominator in one pass. Segment-max is skipped (logits are in
   [-10, 8]; softmax is shift-invariant).
 - Update: combine relations, ELU (shift-invariant form), residual, LayerNorm
   on own rows; AllGather replicates new gene features for the next layer.
 - Tail: batch matmuls contraction-sharded over nodes + one AllReduce, then
   LayerNorm.
"""
import math
import numpy as np

import concourse.bass as bass
import concourse.bacc as bacc
import concourse.tile as tile
from concourse import mybir
from concourse.bass_utils import run_bass_kernel_spmd
from concourse.bass_interp import get_hw_module

F32 = mybir.dt.float32
I16 = mybir.dt.int16
P = 128

FULL_CFG = dict(
    N_GENE=20000, N_CPG=50000, N_MIR=2000, B=64, NCORES=8,
    TILE_NODES=127, IDX_LIM=32768, N_LAYERS=3,
)


def _roundup(x, m):
    return (x + m - 1) // m * m


def wrap_idx16(a):
    """[L] int -> [128, L//16] int16 wrapped layout, replicated 8x across gpsimd cores."""
    L = a.shape[0]
    assert L % 16 == 0
    w = a.reshape(L // 16, 16).T.astype(np.int16)
    return np.ascontiguousarray(np.tile(w, (8, 1)))


def slot_layout(a):
    """[L] -> [128, L//128] f32, token e at [e%128, e//128]."""
    L = a.shape[0]
    assert L % P == 0
    return np.ascontiguousarray(a.reshape(L // P, P).T.astype(np.float32))


def host_prep(inputs, cfg):
    """Build per-core in_maps + the static chunk-count config."""
    NG, NCPG, NMIR = cfg['N_GENE'], cfg['N_CPG'], cfg['N_MIR']
    NC = cfg['NCORES']
    TN = cfg['TILE_NODES']
    NPC = NG // NC
    NT = math.ceil(NPC / TN)
    IDX_LIM = cfg['IDX_LIM']
    NL = cfg['N_LAYERS']

    pa = inputs['params']
    f = lambda x: np.ascontiguousarray(np.asarray(x, np.float32))
    c = f(pa['emb_cpg']); mi = f(pa['emb_mir']); g0 = f(pa['emb_gene'])

    sl = np.arange(NG, dtype=np.int64)
    edges = {
        'cg': (np.asarray(inputs['cg_src'], np.int64), np.asarray(inputs['cg_dst'], np.int64)),
        'mg': (np.asarray(inputs['mg_src'], np.int64), np.asarray(inputs['mg_dst'], np.int64)),
        'gg': (np.concatenate([np.asarray(inputs['gg_src'], np.int64), sl]),
               np.concatenate([np.asarray(inputs['gg_dst'], np.int64), sl])),
    }
    GROUPS = [('cglo', 'cg'), ('cghi', 'cg'), ('mg', 'mg'), ('gg', 'gg')]

    # route / sort / tile / split
    per_core = {}   # (k, grp) -> list over t of (gidx array, slot array)
    for k in range(NC):
        for rel in ['cg', 'mg', 'gg']:
            s, d = edges[rel]
            selm = (d // NPC) == k
            ss, dd = s[selm], d[selm] - k * NPC
            o = np.argsort(dd, kind='stable')
            ss, dd = ss[o], dd[o]
            tid = dd // TN
            slot = dd - tid * TN
            for t in range(NT):
                m = tid == t
                st, so = ss[m], slot[m]
                if rel == 'cg':
                    lo = st < IDX_LIM
                    per_core[(k, 'cglo', t)] = (st[lo], so[lo])
                    per_core[(k, 'cghi', t)] = (st[~lo] - IDX_LIM, so[~lo])
                else:
                    per_core[(k, rel, t)] = (st, so)

    # equalize chunk counts across cores
    nch = {}   # (grp, t) -> chunks of 128
    for grp, rel in GROUPS:
        for t in range(NT):
            mx = max(per_core[(k, grp, t)][0].shape[0] for k in range(NC))
            nch[(grp, t)] = _roundup(max(mx, 1), P) // P

    # pack per-group arrays per core: per segment [gi_wrapped | hdi_wrapped]
    # (both int16) in one array, slot (f32) separate. Segmentation here must
    # match build_program's (t-major, segments of <= SEGCH chunks).
    SEGCH = min(6, max(nch.values()))
    gihd = {k: {} for k in range(NC)}
    slo = {k: {} for k in range(NC)}
    for grp, rel in GROUPS:
        for k in range(NC):
            pkparts, sparts = [], []
            for t in range(NT):
                cnt = nch[(grp, t)] * P
                a, b = per_core[(k, grp, t)]
                ap = np.zeros(cnt, np.int64); ap[:a.shape[0]] = a
                bp = np.full(cnt, 127, np.int64); bp[:b.shape[0]] = b
                hp = np.zeros(cnt, np.int64)
                hp[:b.shape[0]] = b + t * TN   # local dst row in hd table
                n_total = nch[(grp, t)]
                for si in range(math.ceil(n_total / SEGCH)):
                    n = min(SEGCH, n_total - si * SEGCH)
                    sl_ = slice(si * SEGCH * P, si * SEGCH * P + n * P)
                    pkparts.append(wrap_idx16(ap[sl_]))
                    pkparts.append(wrap_idx16(hp[sl_]))
                sparts.append(bp)
            gihd[k][grp] = np.concatenate(pkparts, axis=1)
            slo[k][grp] = slot_layout(np.concatenate(sparts))

    # constants
    iota = np.tile(np.arange(P, dtype=np.float32)[None, :], (P, 1))
    ident = np.eye(P, dtype=np.float32)

    NCPGS, NMIRS = NCPG // NC, NMIR // NC
    xg = f(inputs['xg']); xc = f(inputs['xc']); xm = f(inputs['xm'])
    xgT = np.ascontiguousarray(xg.T) / np.float32(math.sqrt(NG))
    xcT = np.ascontiguousarray(xc.T) / np.float32(math.sqrt(NCPG))
    xmT = np.ascontiguousarray(xm.T) / np.float32(math.sqrt(NMIR))

    in_maps = []
    for k in range(NC):
        m = {
            'cT': f(c.T.reshape(2, 128, NCPG)),
            'miT': f(mi.T.reshape(2, 128, NMIR)),
            'g_full0': g0,
            'g_own0': f(g0[k * NPC:(k + 1) * NPC]),
            'g_own0T': f(g0[k * NPC:(k + 1) * NPC].T.reshape(2, 128, NPC)),
            'iota': iota, 'ident': ident,
            'xgT': f(xgT[k * NPC:(k + 1) * NPC]),
            'xcT': f(xcT[k * NCPGS:(k + 1) * NCPGS]),
            'xmT': f(xmT[k * NMIRS:(k + 1) * NMIRS]),
            'c_rows': f(c[k * NCPGS:(k + 1) * NCPGS]),
            'mi_rows': f(mi[k * NMIRS:(k + 1) * NMIRS]),
            'on_g_g': f(pa['on_gene'][0]), 'on_g_b': f(pa['on_gene'][1]),
            'on_c_g': f(pa['on_cpg'][0]), 'on_c_b': f(pa['on_cpg'][1]),
            'on_m_g': f(pa['on_mir'][0]), 'on_m_b': f(pa['on_mir'][1]),
        }
        for grp, rel in GROUPS:
            m[f'gihd_{grp}'] = gihd[k][grp]
            m[f'slot_{grp}'] = slo[k][grp]
        for L in range(NL):
            lp = pa['layers'][L]
            bias_sum = np.zeros(256, np.float32)
            for rel in ['cg', 'mg', 'gg']:
                rp = lp[rel]
                m[f'Wl_{L}_{rel}'] = f(np.asarray(rp['Wl']).reshape(2, 128, 256))
                m[f'Wr_{L}_{rel}'] = f(np.asarray(rp['Wr']).reshape(2, 128, 256))
                m[f'bl_{L}_{rel}'] = f(rp['bl'])
                m[f'br_{L}_{rel}'] = f(rp['br'])
                m[f'att_{L}_{rel}'] = f(np.asarray(rp['att']).reshape(256))
                bias_sum += np.asarray(rp['bias'], np.float32)
            m[f'bias_sum_{L}'] = bias_sum
            m[f'ln_g_{L}'] = f(lp['ln_g'])
            m[f'ln_b_{L}'] = f(lp['ln_b'])
        in_maps.append(m)

    cfg2 = dict(cfg)
    cfg2['nch'] = nch
    cfg2['GROUPS'] = GROUPS
    cfg2['NPC'] = NPC
    cfg2['NT'] = NT
    return in_maps, cfg2


def bcast_part(ap, parts=P):
    """DRAM [D] vector -> broadcast AP over partitions [parts, D]."""
    return bass.AP(tensor=ap.tensor, offset=ap.offset, ap=[[0, parts]] + list(ap.ap))


def bfree(ap2d, n, axis_pos=1):
    """[P, D] AP -> [P, n, D] with 0-step broadcast middle dim."""
    a = list(ap2d.ap)
    return bass.AP(tensor=ap2d.tensor, offset=ap2d.offset, ap=[a[0], [0, n]] + a[1:])


def build_program(cfg):
    NG, NCPG, NMIR = cfg['N_GENE'], cfg['N_CPG'], cfg['N_MIR']
    NC, B, NL = cfg['NCORES'], cfg['B'], cfg['N_LAYERS']
    TN, NPC, NT = cfg['TILE_NODES'], cfg['NPC'], cfg['NT']
    IDX_LIM = cfg['IDX_LIM']
    nch = cfg['nch']
    GROUPS = cfg['GROUPS']
    NCPGS, NMIRS = NCPG // NC, NMIR // NC
    HDROWS = _roundup(TN * (NT - 1) + P, 64)
    EPS = 1e-5
    DEN_EPS = 1e-10

    nc = bacc.Bacc("TRN2", target_bir_lowering=False, debug=False, num_devices=NC)
    dt = lambda n, s, d=F32, k="ExternalInput": nc.dram_tensor(n, s, d, kind=k).ap()

    cT = dt('cT', [2, 128, NCPG]); miT = dt('miT', [2, 128, NMIR])
    g_full0 = dt('g_full0', [NG, 256])
    g_own0 = dt('g_own0', [NPC, 256]); g_own0T = dt('g_own0T', [2, 128, NPC])
    iota_t = dt('iota', [P, P]); ident_t = dt('ident', [P, P])
    xgT = dt('xgT', [NPC, B]); xcT = dt('xcT', [NCPGS, B]); xmT = dt('xmT', [NMIRS, B])
    c_rows = dt('c_rows', [NCPGS, 256]); mi_rows = dt('mi_rows', [NMIRS, 256])
    on_ap = {n: dt(n, [256]) for n in ['on_g_g', 'on_g_b', 'on_c_g', 'on_c_b', 'on_m_g', 'on_m_b']}
    pk_ap, slot_ap = {}, {}
    for grp, rel in GROUPS:
        T = sum(nch[(grp, t)] for t in range(NT)) * P
        pk_ap[grp] = dt(f'gihd_{grp}', [128, T // 8], I16)
        slot_ap[grp] = dt(f'slot_{grp}', [128, T // P])
    wap = {}
    for L in range(NL):
        for rel in ['cg', 'mg', 'gg']:
            for w in ['Wl', 'Wr']:
                wap[(w, L, rel)] = dt(f'{w}_{L}_{rel}', [2, 128, 256])
            for v in ['bl', 'br', 'att']:
                wap[(v, L, rel)] = dt(f'{v}_{L}_{rel}', [256])
        for v in ['bias_sum', 'ln_g', 'ln_b']:
            wap[(v, L)] = dt(f'{v}_{L}', [256])

    # internal DRAM (per-layer table buffers so later layers' tables can be
    # computed early without write-after-read hazards on gathers)
    it = lambda n, s, d=F32: nc.dram_tensor(n, s, d).ap()
    hs_tabs = [{'cg': it(f'hs_cg_{L}', [NCPG, 256]), 'mg': it(f'hs_mg_{L}', [NMIR, 256]),
                'gg': it(f'hs_gg_{L}', [NG, 256])} for L in range(NL)]
    hd_tabs = [{r: it(f'hd_{r}_{L}', [HDROWS, 256]) for r in ['cg', 'mg', 'gg']}
               for L in range(NL)]
    g_own_buf = [g_own0] + [it(f'g_own{L + 1}', [NPC, 256]) for L in range(NL)]
    g_ownT_buf = [g_own0T] + [it(f'g_ownT{L + 1}', [2, 128, NPC]) for L in range(NL - 1)]
    ag_out = [None] + [nc.dram_tensor(f'ag_out{L + 1}', [NG, 256], F32, addr_space="Shared").ap()
                       for L in range(NL - 1)]
    ar_in = it('ar_in', [B, 3, 256])
    ar_out = nc.dram_tensor('ar_out', [B, 3, 256], F32, addr_space="Shared").ap()
    zout = dt('zout', [3, B, 256], F32, "ExternalOutput")

    SEGCH = min(6, max(nch.values()))

    with tile.TileContext(nc) as tc:
        with tc.tile_pool(name="consts", bufs=1) as consts, \
             tc.tile_pool(name="wpool", bufs=1) as wpool, \
             tc.tile_pool(name="tabs", bufs=3) as tabs, \
             tc.tile_pool(name="edges", bufs=4) as epool, \
             tc.tile_pool(name="upd", bufs=2) as upool, \
             tc.tile_pool(name="opart", bufs=1) as opool, \
             tc.tile_pool(name="psB", bufs=3, space="PSUM") as psB, \
             tc.tile_pool(name="psRc", bufs=2, space="PSUM") as psRc, \
             tc.tile_pool(name="psR", bufs=1, space="PSUM") as psR, \
             tc.tile_pool(name="psR2", bufs=2, space="PSUM") as psR2:

            iota_sb = consts.tile([P, P], F32)
            nc.sync.dma_start(out=iota_sb[:], in_=iota_t)
            ident_sb = consts.tile([P, P], F32)
            nc.sync.dma_start(out=ident_sb[:], in_=ident_t)
            eps_sb = consts.tile([P, 1], F32)
            nc.vector.memset(eps_sb[:], EPS)
            zero_sb = consts.tile([P, 256], F32)
            nc.vector.memset(zero_sb[:], 0.0)
            # zero hd table pad rows once
            if HDROWS > NPC:
                for L_ in range(NL):
                    for r in ['cg', 'mg', 'gg']:
                        nc.sync.dma_start(out=hd_tabs[L_][r][NPC:HDROWS, :],
                                          in_=zero_sb[:HDROWS - NPC, :])

            _rep_n = [0]

            def rep_tile(vec_ap, tag=None):
                if tag is None:
                    _rep_n[0] += 1
                    tag = f"rep{_rep_n[0]}"
                t = wpool.tile([P, 256], F32, tag=tag)
                nc.sync.dma_start(out=t[:], in_=bcast_part(vec_ap))
                return t

            TPB = 4  # row-tiles per DMA block in table builds

            def table_rowmajor(dst_ap, n_rows, srcT_ap, W_ap, bias_rep, src_is_dram):
                """dst[n_rows,256] = srcT.T @ W + bias. srcT: [2,128,n_rows] DRAM."""
                nblk = math.ceil(n_rows / (TPB * P))
                for blk in range(nblk):
                    r0 = blk * TPB * P
                    wb = min(TPB * P, n_rows - r0)
                    nt_ = math.ceil(wb / P)
                    lt = tabs.tile([P, 2, TPB * P], F32, tag="tab_lhsT")
                    nc.sync.dma_start(
                        out=lt[:, :, :wb],
                        in_=srcT_ap.rearrange("k p n -> p k n")[:, :, r0:r0 + wb])
                    ot = tabs.tile([P, TPB, 256], F32, tag="tab_out")
                    for b in range(nt_):
                        w = min(P, wb - b * P)
                        ps = psB.tile([P, 256], F32, space="PSUM", tag="psB")
                        for kk in range(2):
                            nc.tensor.matmul(ps[:w, :], lhsT=lt[:, kk, b * P:b * P + w],
                                             rhs=W_ap[:, kk, :],
                                             start=(kk == 0), stop=(kk == 1), skip_group_check=True)
                        nc.vector.tensor_tensor(out=ot[:w, b, :], in0=ps[:w, :], in1=bias_rep[:w, :],
                                                op=mybir.AluOpType.add)
                    if wb == TPB * P:
                        nc.scalar.dma_start(
                            out=dst_ap[r0:r0 + wb, :].rearrange("(b p) n -> p b n", p=P),
                            in_=ot[:, :, :])
                    else:
                        for b in range(nt_):
                            w = min(P, wb - b * P)
                            nc.scalar.dma_start(out=dst_ap[r0 + b * P:r0 + b * P + w, :],
                                                in_=ot[:w, b, :])

            def table_from_rows(dst_ap, n_rows, src_rows_ap, W_ap, bias_rep):
                """dst[n_rows,256] = src_rows @ W + bias, transposing tiles on PE."""
                nblk = math.ceil(n_rows / (TPB * P))
                for blk in range(nblk):
                    r0 = blk * TPB * P
                    wb = min(TPB * P, n_rows - r0)
                    nt_ = math.ceil(wb / P)
                    rt = tabs.tile([P, TPB, 256], F32, tag="tab_rows")
                    if wb % P == 0:
                        nc.sync.dma_start(
                            out=rt[:, :nt_, :],
                            in_=src_rows_ap[r0:r0 + wb, :].rearrange("(b p) n -> p b n", p=P))
                    else:
                        for b in range(nt_):
                            w = min(P, wb - b * P)
                            nc.sync.dma_start(out=rt[:w, b, :],
                                              in_=src_rows_ap[r0 + b * P:r0 + b * P + w, :])
                    ot = tabs.tile([P, TPB, 256], F32, tag="tab_out")
                    for b in range(nt_):
                        w = min(P, wb - b * P)
                        gt = tabs.tile([P, 2, P], F32, tag="tab_gT")
                        for kk in range(2):
                            pst = psB.tile([P, P], F32, space="PSUM", tag="psB")
                            nc.tensor.transpose(out=pst[:, :w], in_=rt[:w, b, kk * 128:(kk + 1) * 128],
                                                identity=ident_sb[:w, :w])
                            nc.scalar.copy(out=gt[:, kk, :w], in_=pst[:, :w])
                        ps = psB.tile([P, 256], F32, space="PSUM", tag="psB")
                        for kk in range(2):
                            nc.tensor.matmul(ps[:w, :], lhsT=gt[:, kk, :w], rhs=W_ap[:, kk, :],
                                             start=(kk == 0), stop=(kk == 1), skip_group_check=True)
                        nc.vector.tensor_tensor(out=ot[:w, b, :], in0=ps[:w, :], in1=bias_rep[:w, :],
                                                op=mybir.AluOpType.add)
                    if wb == TPB * P:
                        nc.scalar.dma_start(
                            out=dst_ap[r0:r0 + wb, :].rearrange("(b p) n -> p b n", p=P),
                            in_=ot[:, :, :])
                    else:
                        for b in range(nt_):
                            w = min(P, wb - b * P)
                            nc.scalar.dma_start(out=dst_ap[r0 + b * P:r0 + b * P + w, :],
                                                in_=ot[:w, b, :])

            # ---------------- layers ----------------
            for L in range(NL):
                g_full = g_full0 if L == 0 else ag_out[L]
                g_ownT = g_ownT_buf[L]

                att_rep, W_sb = {}, {}
                for rel in ['cg', 'mg', 'gg']:
                    att_rep[rel] = rep_tile(wap[('att', L, rel)], tag=f"att_{rel}")
                    for w in ['Wl', 'Wr']:
                        t_ = wpool.tile([P, 2, 256], F32, tag=f"{w}_{rel}")
                        nc.sync.dma_start(out=t_[:], in_=wap[(w, L, rel)].rearrange("k p n -> p k n"))
                        W_sb[(w, rel)] = t_
                bias_sum_rep = rep_tile(wap[('bias_sum', L)], tag="bias_sum")
                lng_rep = rep_tile(wap[('ln_g', L)], tag="lng")
                lnb_rep = rep_tile(wap[('ln_b', L)], tag="lnb")
                bl_rep = {rel: rep_tile(wap[('bl', L, rel)], tag=f"bl_{rel}") for rel in ['cg', 'mg', 'gg']}
                br_rep = {rel: rep_tile(wap[('br', L, rel)], tag=f"br_{rel}") for rel in ['cg', 'mg', 'gg']}

                hs_tab, hd_tab = hs_tabs[L], hd_tabs[L]
                # hd tables first (edge phase A needs them), then cg/mg hs tables
                if not cfg.get('skip_tables'):
                    for rel in ['cg', 'mg', 'gg']:
                        table_rowmajor(hd_tab[rel], NPC, g_ownT, W_sb[('Wr', rel)], br_rep[rel], True)
                    table_rowmajor(hs_tab['cg'], NCPG, cT, W_sb[('Wl', 'cg')], bl_rep['cg'], True)
                    table_rowmajor(hs_tab['mg'], NMIR, miT, W_sb[('Wl', 'mg')], bl_rep['mg'], True)

                gsrc = {'cglo': hs_tab['cg'][0:min(IDX_LIM, NCPG), :],
                        'cghi': hs_tab['cg'][IDX_LIM:NCPG, :] if NCPG > IDX_LIM else None,
                        'mg': hs_tab['mg'], 'gg': hs_tab['gg']}

                goff = {grp: 0 for grp, _ in GROUPS}
                pkoff = {grp: 0 for grp, _ in GROUPS}
                psum_r = {}      # (rel, t) -> live psum handle
                started = {}     # (rel, t) -> bool

                def do_group(grp, rel, t, last_of_rel):
                    """Emit gathers + edge math + segment matmuls for (grp, t)."""
                    n_total = nch[(grp, t)]
                    if n_total == 0 or gsrc[grp] is None:
                        return
                    nseg = math.ceil(n_total / SEGCH)
                    for si in range(nseg):
                        n = min(SEGCH, n_total - si * SEGCH)
                        last_seg = si == nseg - 1
                        off = goff[grp]
                        goff[grp] = off + n
                        pko = pkoff[grp]
                        pkoff[grp] = pko + n * 16
                        ne = n * P
                        pksb = epool.tile([P, SEGCH * 16], I16, tag="pk", name=f"pk_{L}_{t}_{grp}_{si}")
                        nc.sync.dma_start(out=pksb[:, :n * 16],
                                          in_=pk_ap[grp][:, pko:pko + n * 16])
                        gixsb = pksb[:, 0:n * 8]
                        hdixsb = pksb[:, n * 8:n * 16]
                        slsb = epool.tile([P, SEGCH], F32, tag="slot", name=f"sl_{L}_{t}_{grp}_{si}")
                        nc.sync.dma_start(out=slsb[:, :n], in_=slot_ap[grp][:, off:off + n])
                        hs_big = epool.tile([P, SEGCH, 256], F32, tag="hs_big", name=f"hs_{L}_{t}_{grp}_{si}")
                        hd_big = epool.tile([P, SEGCH, 256], F32, tag="hd_big", name=f"hd_{L}_{t}_{grp}_{si}")
                        if cfg.get('no_gather'):
                            nc.vector.memset(hs_big[:, :n, :], 0.01)
                            nc.vector.memset(hd_big[:, :n, :], 0.01)
                        else:
                            nc.gpsimd.dma_gather(
                                out_ap=hs_big[:, :n, :], in_ap=gsrc[grp], idxs_ap=gixsb,
                                num_idxs=ne, num_idxs_reg=ne, elem_size=256)
                            nc.gpsimd.dma_gather(
                                out_ap=hd_big[:, :n, :], in_ap=hd_tab[rel], idxs_ap=hdixsb,
                                num_idxs=ne, num_idxs_reg=ne, elem_size=256)
                        oh_w = epool.tile([P, SEGCH, P], F32, tag="oh_w", name=f"oh_{L}_{t}_{grp}_{si}")
                        slotb = bass.AP(tensor=slsb.tensor, offset=slsb[:].offset,
                                        ap=[slsb[:].ap[0], [1, n], [0, P]])
                        iotab = bass.AP(tensor=iota_sb.tensor, offset=iota_sb[:].offset,
                                        ap=[iota_sb[:].ap[0], [0, n], [1, P]])
                        nc.vector.tensor_tensor(out=oh_w[:, :n, :], in0=slotb, in1=iotab,
                                                op=mybir.AluOpType.is_equal)
                        # s = hs + hd computed in place in hd_big
                        s_w = hd_big
                        nc.vector.tensor_tensor(out=s_w[:, :n, :], in0=hs_big[:, :n, :],
                                                in1=hd_big[:, :n, :], op=mybir.AluOpType.add)
                        pay = epool.tile([P, SEGCH, 264], F32, tag="pay", name=f"pay_{L}_{t}_{grp}_{si}")
                        if cfg.get('act_lrelu', False):
                            # leaky-relu fused on the scalar engine
                            nc.scalar.activation(out=s_w[:, :n, :], in_=s_w[:, :n, :],
                                                 func=mybir.ActivationFunctionType.Lrelu,
                                                 alpha=0.2)
                        else:
                            # pay[:, :, 0:256] doubles as the 0.2*s scratch
                            nc.scalar.mul(out=pay[:, :n, 0:256], in_=s_w[:, :n, :], mul=0.2)
                            nc.vector.tensor_tensor(out=s_w[:, :n, :], in0=s_w[:, :n, :],
                                                    in1=pay[:, :n, 0:256], op=mybir.AluOpType.max)
                        nc.vector.tensor_tensor(out=s_w[:, :n, :], in0=s_w[:, :n, :],
                                                in1=bfree(att_rep[rel][:], n),
                                                op=mybir.AluOpType.mult)
                        e_w = epool.tile([P, SEGCH, 8], F32, tag="e_w", name=f"e_{L}_{t}_{grp}_{si}")
                        nc.vector.reduce_sum(out=e_w[:, :n, :],
                                             in_=s_w[:, :n, :].rearrange("p c (h k) -> p c h k", k=32),
                                             axis=mybir.AxisListType.X)
                        ex_w = epool.tile([P, SEGCH, 8], F32, tag="ex_w", name=f"ex_{L}_{t}_{grp}_{si}")
                        nc.scalar.activation(out=ex_w[:, :n, :], in_=e_w[:, :n, :],
                                             func=mybir.ActivationFunctionType.Exp)
                        exb = bass.AP(tensor=ex_w.tensor, offset=ex_w[:].offset,
                                      ap=[ex_w[:].ap[0], [8, n], [1, 8], [0, 32]])
                        nc.vector.tensor_tensor(
                            out=pay[:, :n, 0:256].rearrange("p c (h k) -> p c h k", k=32),
                            in0=hs_big[:, :n, :].rearrange("p c (h k) -> p c h k", k=32),
                            in1=exb, op=mybir.AluOpType.mult)
                        nc.vector.tensor_copy(out=pay[:, :n, 256:264], in_=ex_w[:, :n, :])
                        for j in range(n):
                            st = not started[(rel, t)]
                            started[(rel, t)] = True
                            sp = last_of_rel and last_seg and (j == n - 1)
                            nc.tensor.matmul(psum_r[(rel, t)][:], lhsT=oh_w[:, j, :], rhs=pay[:, j, :],
                                             start=st, stop=sp, skip_group_check=True)

                def rel_norm(rel, t, dst_tile, accumulate):
                    """dst (+)= psum_r[(rel,t)] msg / (den + eps)."""
                    ps = psum_r[(rel, t)]
                    den = upool.tile([P, 8], F32, tag="den", name=f"den_{L}_{t}_{rel}")
                    nc.vector.tensor_scalar(out=den[:], in0=ps[:, 256:264],
                                            scalar1=DEN_EPS, scalar2=None,
                                            op0=mybir.AluOpType.add)
                    rec = upool.tile([P, 8], F32, tag="rec", name=f"rec_{L}_{t}_{rel}")
                    nc.vector.reciprocal(out=rec[:], in_=den[:])
                    recb = bass.AP(tensor=rec.tensor, offset=rec[:].offset,
                                   ap=[rec[:].ap[0], [1, 8], [0, 32]])
                    if not accumulate:
                        nc.vector.tensor_tensor(
                            out=dst_tile[:].rearrange("p (h k) -> p h k", k=32),
                            in0=ps[:, 0:256].rearrange("p (h k) -> p h k", k=32),
                            in1=recb, op=mybir.AluOpType.mult)
                    else:
                        tmp = upool.tile([P, 256], F32, tag="tmp", name=f"tmp_{L}_{t}_{rel}")
                        nc.vector.tensor_tensor(
                            out=tmp[:].rearrange("p (h k) -> p h k", k=32),
                            in0=ps[:, 0:256].rearrange("p (h k) -> p h k", k=32),
                            in1=recb, op=mybir.AluOpType.mult)
                        nc.vector.tensor_tensor(out=dst_tile[:], in0=dst_tile[:], in1=tmp[:],
                                                op=mybir.AluOpType.add)

                o_part = {}
                if not cfg.get('skip_edges'):
                    # ---- phase A: cg + mg edges (overlaps with nothing yet; gg
                    # table below overlaps with this) ----
                    for t in range(NT):
                        psum_r[('cg', t)] = psRc.tile([P, 264], F32, space="PSUM",
                                                      tag="seg_cg", name=f"ps_cg_{L}_{t}")
                        psum_r[('mg', t)] = psR.tile([P, 264], F32, space="PSUM",
                                                     tag="seg_mg", name=f"ps_mg_{L}_{t}")
                        started[('cg', t)] = started[('mg', t)] = False
                        do_group('cglo', 'cg', t, last_of_rel=(nch[('cghi', t)] == 0 or gsrc['cghi'] is None))
                        do_group('cghi', 'cg', t, last_of_rel=True)
                        op = opool.tile([P, 256], F32, tag=f"opart_{t}", name=f"opart_{L}_{t}")
                        rel_norm('cg', t, op, accumulate=False)
                        do_group('mg', 'mg', t, last_of_rel=True)
                        rel_norm('mg', t, op, accumulate=True)
                        o_part[t] = op

                # gg hs table — emitted here so it schedules alongside phase A
                if not cfg.get('skip_tables'):
                    table_from_rows(hs_tab['gg'], NG, g_full, W_sb[('Wl', 'gg')], bl_rep['gg'])

                for t in range(0 if not cfg.get('skip_edges') else NT, NT):
                    nodes_t = min(TN, NPC - TN * t)
                    # ---- phase B: gg edges ----
                    psum_r[('gg', t)] = psR2.tile([P, 264], F32, space="PSUM",
                                                  tag="seg_gg", name=f"ps_gg_{L}_{t}")
                    started[('gg', t)] = False
                    do_group('gg', 'gg', t, last_of_rel=True)
                    o_acc = upool.tile([P, 256], F32, tag="o_acc", name=f"oacc_{L}_{t}")
                    rel_norm('gg', t, o_acc, accumulate=False)
                    nc.vector.tensor_tensor(out=o_acc[:], in0=o_acc[:], in1=o_part[t][:],
                                            op=mybir.AluOpType.add)
                    nc.vector.tensor_tensor(out=o_acc[:], in0=o_acc[:], in1=bias_sum_rep[:],
                                            op=mybir.AluOpType.add)
                    # elu(o) + 1 = relu(o) + exp(min(o,0)); the -1 shift cancels in LN
                    neg = upool.tile([P, 256], F32, tag="neg")
                    nc.vector.tensor_scalar(out=neg[:], in0=o_acc[:], scalar1=0.0, scalar2=None,
                                            op0=mybir.AluOpType.min)
                    en = upool.tile([P, 256], F32, tag="en")
                    nc.scalar.activation(out=en[:], in_=neg[:], func=mybir.ActivationFunctionType.Exp)
                    nc.vector.tensor_scalar(out=o_acc[:], in0=o_acc[:], scalar1=0.0, scalar2=None,
                                            op0=mybir.AluOpType.max)
                    nc.vector.tensor_tensor(out=o_acc[:], in0=o_acc[:], in1=en[:],
                                            op=mybir.AluOpType.add)
                    gold = upool.tile([P, 256], F32, tag="gold")
                    nc.sync.dma_start(out=gold[:nodes_t, :],
                                      in_=g_own_buf[L][TN * t:TN * t + nodes_t, :])
                    nc.vector.tensor_tensor(out=o_acc[:nodes_t, :], in0=o_acc[:nodes_t, :],
                                            in1=gold[:nodes_t, :], op=mybir.AluOpType.add)
                    # LayerNorm
                    stats = upool.tile([P, 6], F32, tag="stats")
                    nc.vector.bn_stats(out=stats[:nodes_t, :], in_=o_acc[:nodes_t, :])
                    mv = upool.tile([P, 2], F32, tag="mv")
                    nc.vector.bn_aggr(out=mv[:nodes_t, :], in_=stats[:nodes_t, :])
                    sd = upool.tile([P, 1], F32, tag="sd")
                    nc.scalar.activation(out=sd[:nodes_t, :], in_=mv[:nodes_t, 1:2],
                                         func=mybir.ActivationFunctionType.Sqrt,
                                         bias=eps_sb[:nodes_t, :])
                    rs = upool.tile([P, 1], F32, tag="rs")
                    nc.vector.reciprocal(out=rs[:nodes_t, :], in_=sd[:nodes_t, :])
                    nc.vector.tensor_scalar(out=o_acc[:nodes_t, :], in0=o_acc[:nodes_t, :],
                                            scalar1=mv[:nodes_t, 0:1], scalar2=rs[:nodes_t, :],
                                            op0=mybir.AluOpType.subtract,
                                            op1=mybir.AluOpType.mult)
                    nc.vector.tensor_tensor(out=o_acc[:nodes_t, :], in0=o_acc[:nodes_t, :],
                                            in1=lng_rep[:nodes_t, :], op=mybir.AluOpType.mult)
                    nc.vector.tensor_tensor(out=o_acc[:nodes_t, :], in0=o_acc[:nodes_t, :],
                                            in1=lnb_rep[:nodes_t, :], op=mybir.AluOpType.add)
                    nc.sync.dma_start(out=g_own_buf[L + 1][TN * t:TN * t + nodes_t, :],
                                      in_=o_acc[:nodes_t, :])
                    if L < NL - 1:
                        gtn = upool.tile([P, 2, P], F32, tag="gtn")
                        for kk in range(2):
                            pst = psB.tile([P, P], F32, space="PSUM", tag="psB")
                            nc.tensor.transpose(out=pst[:, :nodes_t],
                                                in_=o_acc[:nodes_t, kk * 128:(kk + 1) * 128],
                                                identity=ident_sb[:nodes_t, :nodes_t])
                            nc.scalar.copy(out=gtn[:, kk, :nodes_t], in_=pst[:, :nodes_t])
                            nc.sync.dma_start(
                                out=g_ownT_buf[L + 1].rearrange("k p n -> p k n")[:, kk, TN * t:TN * t + nodes_t],
                                in_=gtn[:, kk, :nodes_t])

                if L < NL - 1:
                    if cfg.get('no_collective'):
                        nc.sync.dma_start(out=ag_out[L + 1][0:NPC, :], in_=g_own_buf[L + 1])
                    else:
                        nc.gpsimd.collective_compute(
                            "AllGather", mybir.AluOpType.bypass,
                            replica_groups=[list(range(NC))],
                            ins=[g_own_buf[L + 1].opt()], outs=[ag_out[L + 1].opt()])

            # ---------------- tail ----------------
            def tail_mm(xT_ap, rows_ap, n_rows):
                ps = psB.tile([P, 256], F32, space="PSUM", tag="psB")
                ntile = math.ceil(n_rows / P)
                for t in range(ntile):
                    w = min(P, n_rows - t * P)
                    lt = upool.tile([P, B], F32, tag="tail_lt")
                    nc.sync.dma_start(out=lt[:w, :], in_=xT_ap[t * P:t * P + w, :])
                    rt = upool.tile([P, 256], F32, tag="tail_rt")
                    nc.sync.dma_start(out=rt[:w, :], in_=rows_ap[t * P:t * P + w, :])
                    nc.tensor.matmul(ps[:B, :], lhsT=lt[:w, :B], rhs=rt[:w, :],
                                     start=(t == 0), stop=(t == ntile - 1), skip_group_check=True)
                return ps

            zcat = upool.tile([P, 3, 256], F32, tag="zcat")
            for i, (xT_ap, rows_ap, n_rows) in enumerate([
                    (xgT, g_own_buf[NL], NPC), (xcT, c_rows, NCPGS), (xmT, mi_rows, NMIRS)]):
                ps = tail_mm(xT_ap, rows_ap, n_rows)
                nc.scalar.copy(out=zcat[:B, i, :], in_=ps[:B, :])
            nc.sync.dma_start(out=ar_in, in_=zcat[:B, :, :])
            if cfg.get('no_collective'):
                nc.sync.dma_start(out=ar_out, in_=ar_in)
            else:
                nc.gpsimd.collective_compute(
                    "AllReduce", mybir.AluOpType.add,
                    replica_groups=[list(range(NC))],
                    ins=[ar_in.opt()], outs=[ar_out.opt()])
            zs = upool.tile([P, 3, 256], F32, tag="zs")
            nc.sync.dma_start(out=zs[:B, :, :], in_=ar_out)
            for i, (gn, bn) in enumerate([('on_g_g', 'on_g_b'), ('on_c_g', 'on_c_b'),
                                          ('on_m_g', 'on_m_b')]):
                x = zs[:B, i, :]
                stats = upool.tile([P, 6], F32, tag="stats")
                nc.vector.bn_stats(out=stats[:B, :], in_=x)
                mv = upool.tile([P, 2], F32, tag="mv")
                nc.vector.bn_aggr(out=mv[:B, :], in_=stats[:B, :])
                sd = upool.tile([P, 1], F32, tag="sd")
                nc.scalar.activation(out=sd[:B, :], in_=mv[:B, 1:2],
                                     func=mybir.ActivationFunctionType.Sqrt, bias=eps_sb[:B, :])
                rs = upool.tile([P, 1], F32, tag="rs")
                nc.vector.reciprocal(out=rs[:B, :], in_=sd[:B, :])
                nc.vector.tensor_scalar(out=x, in0=x, scalar1=mv[:B, 0:1], scalar2=rs[:B, :],
                                        op0=mybir.AluOpType.subtract, op1=mybir.AluOpType.mult)
                grt = rep_tile(on_ap[gn])
                brt = rep_tile(on_ap[bn])
                nc.vector.tensor_tensor(out=x, in0=x, in1=grt[:B, :], op=mybir.AluOpType.mult)
                nc.vector.tensor_tensor(out=x, in0=x, in1=brt[:B, :], op=mybir.AluOpType.add)
                nc.sync.dma_start(out=zout[i, :, :], in_=zs[:B, i, :])

    nc.compile()
    return nc


_last_results = None


def build(inputs, cfg_overrides=None):
    cfg = dict(FULL_CFG)
    if cfg_overrides:
        cfg.update(cfg_overrides)
    in_maps, cfg = host_prep(inputs, cfg)
    nc = build_program(cfg)
    nc.m = get_hw_module(nc.m)
    return nc, in_maps, cfg


def kernel(**inputs):
    global _last_results
    nc, in_maps, cfg = build(inputs)
    res = run_bass_kernel_spmd(nc, in_maps, core_ids=list(range(cfg['NCORES'])))
    _last_results = res
    z = np.asarray(res.results[0]['zout']).reshape(3, cfg['B'], 256)
    return np.ascontiguousarray(z[0]), np.ascontiguousarray(z[1]), np.ascontiguousarray(z[2])
